# revision 6
# baseline (speedup 1.0000x reference)
"""Trainium2 Bass kernel for nn_CustomWeightedTensorProduct (e3nn-style weighted
tensor product, 5 paths, per-edge weights).

Strategy (pure data-parallel over the edge/batch dim, 8 cores):
  - Pad Z=100000 -> 100352 = 8 * 12544; each core processes 12544 edges.
  - Per core: 7 tiles of 1792 edges laid out as [128 partitions x 14 edges].
  - Math is factored so each weight element is touched once:
      out0 = (sw0 . s1_0) * s2_0 / sqrt32 + (sw3 . b) / (sqrt3*sqrt32)
               with b[u] = <s1_1[u], s2_1>
      out1[w,i] = ((sw1 . s1_0)[w] * s2_1[i]
                   + (sw2 . s1_1[:,i])[w] * s2_0
                   + cross((sw4 . s1_1), s2_1)[w,i] / sqrt2) / sqrt48
  - Contractions over u run as bf16 tensor_tensor multiplies (DVE 2x mode)
    against Act-engine broadcast-expanded operands, followed by a binary tree
    of bf16 adds (2x mode). Path-3 contraction and the combine stage run on
    GPSIMD; expansions/casts run on the Scalar engine. Final combine in fp32.
"""

import sys

if "/opt/trn_rl_repo" not in sys.path:
    sys.path.insert(0, "/opt/trn_rl_repo")

import numpy as np

Z_FULL = 100000
N_CORES = 8
P = 128
C = 14                      # edges per partition per tile
TILE_E = P * C              # 1792
N_TILES = 7
ZC = TILE_E * N_TILES       # 12544 edges per core
Z_PAD = ZC * N_CORES        # 100352

SQRT2 = 2.0 ** 0.5
SQRT3 = 3.0 ** 0.5
K0 = 1.0 / (32.0 ** 0.5)            # out0 scale
K3 = K0 / SQRT3                     # sw3 path scale
K1 = 1.0 / (48.0 ** 0.5)            # out1 scale
KD = K1 / SQRT2                     # cross path scale

USE_BF16 = True


def build_bass(n_tiles=N_TILES, repeat=1):
    import concourse.bass as bass  # noqa: F401
    import concourse.bacc as bacc
    import concourse.mybir as mybir
    from concourse.tile import TileContext

    zc = TILE_E * n_tiles
    f32 = mybir.dt.float32
    bf = mybir.dt.bfloat16 if USE_BF16 else f32
    ADD = mybir.AluOpType.add
    MUL = mybir.AluOpType.mult
    SUB = mybir.AluOpType.subtract
    AX = mybir.AxisListType.X

    nc = bacc.Bacc(None, target_bir_lowering=False)
    # w/x1 are pre-cast to bf16 on the host: halves HBM traffic and keeps
    # all loads on the fast HWDGE path (no SWDGE cast descriptors).
    x1_d = nc.dram_tensor("x1", [zc, 64], bf, kind="ExternalInput")
    x2_d = nc.dram_tensor("x2", [zc, 4], f32, kind="ExternalInput")
    w_d = nc.dram_tensor("w", [zc, 1280], bf, kind="ExternalInput")
    out_d = nc.dram_tensor("out", [zc, 64], f32, kind="ExternalOutput")

    cast_dma = nc.sync

    with TileContext(nc) as tc:
        with (
            tc.tile_pool(name="io", bufs=2) as pio,
            tc.tile_pool(name="mid", bufs=1) as pm,
            tc.tile_pool(name="small", bufs=2) as ps,
            tc.tile_pool(name="small1", bufs=1) as ps1,
        ):
            for it in range(n_tiles * repeat):
                t = it % n_tiles
                r0 = t * TILE_E
                wv = w_d[r0:r0 + TILE_E, :].rearrange("(p c) d -> p (c d)", p=P)
                x1v = x1_d[r0:r0 + TILE_E, :].rearrange("(p c) d -> p (c d)", p=P)
                x2v = x2_d[r0:r0 + TILE_E, :].rearrange("(p c) d -> p (c d)", p=P)
                outv = out_d[r0:r0 + TILE_E, :].rearrange("(p c) d -> p (c d)", p=P)

                # ---- loads ----
                Wt = pio.tile([P, C * 1280], bf)
                X1t = pio.tile([P, C * 64], bf)
                X2t = pio.tile([P, C * 4], f32)
                OUTt = pio.tile([P, C * 64], f32)
                cast_dma.dma_start(X1t[:], x1v)
                nc.gpsimd.dma_start(X2t[:], x2v)
                cast_dma.dma_start(Wt[:], wv)

                x1b = X1t.rearrange("p (c d) -> p c d", d=64)
                s10 = x1b[:, :, 0:16]                                       # (c,16)
                s11 = x1b[:, :, 16:64].rearrange("p c (u i) -> p c u i", i=3)

                # ---- x2-derived scale tiles. Act handles only contiguous
                # copies (stride-0 reads on Act measured ~13x slow); the
                # broadcast-shaped X2D lives on GPSIMD which handles them fine.
                X2bf = ps.tile([P, C * 4], bf)
                nc.scalar.copy(X2bf[:], X2t[:])
                x2bc = X2bf.rearrange("p (c f) -> p c f", f=4)

                X2A = ps.tile([P, C], bf)       # s2_0 * K0
                nc.scalar.mul(X2A[:], x2bc[:, :, 0], K0)
                X2BC = ps.tile([P, C * 4], bf)  # x2 * K1 (col0=B, cols1:3=C)
                nc.scalar.mul(X2BC[:], X2bf[:], K1)
                x2bcv = X2BC.rearrange("p (c f) -> p c f", f=4)
                X2D = ps.tile([P, C * 6], bf)   # s2_1 * KD, duplicated twice
                nc.gpsimd.tensor_scalar(
                    X2D.rearrange("p (c r k) -> p c r k", r=2, k=3),
                    x2bc[:, :, 1:4].unsqueeze(2).broadcast_to([P, C, 2, 3]),
                    float(KD), None, MUL)

                # ---- b[u] = <s1_1[u,:], s2_1> (DVE mult + reduce) ----
                Bp = pm.tile([P, C * 48], bf)
                Bpv = Bp.rearrange("p (c u i) -> p c u i", u=16, i=3)
                nc.vector.tensor_tensor(
                    Bpv, s11,
                    x2bc[:, :, 1:4].unsqueeze(2).broadcast_to([P, C, 16, 3]), MUL,
                )
                bT = ps.tile([P, C * 16], f32)
                bTv = bT.rearrange("p (c u) -> p c u", u=16)
                nc.vector.tensor_reduce(bTv, Bpv, axis=AX, op=ADD)

                # ---- broadcast expansions on GPSIMD (handles stride-0 reads
                # at ~line rate, unlike Act/DVE) ----
                E1 = pm.tile([P, C * 256], bf)
                E1v = E1.rearrange("p (c u w) -> p c u w", u=16, w=16)
                nc.gpsimd.tensor_copy(
                    E1v, s10.unsqueeze(3).broadcast_to([P, C, 16, 16]))
                Eb = pm.tile([P, C * 256], bf)
                Ebv = Eb.rearrange("p (c u w) -> p c u w", u=16, w=16)
                nc.gpsimd.tensor_copy(
                    Ebv, bTv.unsqueeze(3).broadcast_to([P, C, 16, 16]))
                Es = pm.tile([P, C * 768], bf)
                Esv = Es.rearrange("p (c i u w) -> p c i u w", i=3, u=16, w=16)
                for i in range(3):
                    nc.gpsimd.tensor_copy(
                        Esv[:, :, i],
                        s11[:, :, :, i].unsqueeze(3).broadcast_to([P, C, 16, 16]),
                    )

                wt = Wt.rearrange("p (c q) -> p c q", q=1280)

                # shared DVE scratch
                Pt = pm.tile([P, C * 768], bf)
                Ptv = Pt.rearrange("p (c x) -> p c x", x=768)
                At = pm.tile([P, C * 384], bf)
                Atv = At.rearrange("p (c x) -> p c x", x=384)
                Bt = pm.tile([P, C * 192], bf)
                Btv = Bt.rearrange("p (c x) -> p c x", x=192)
                Ct = pm.tile([P, C * 96], bf)
                Ctv = Ct.rearrange("p (c x) -> p c x", x=96)

                def dve_contract(win, ein, nblk, tout):
                    # Multiply stage split per block so every operand AP is
                    # stride-affine with no zero strides (broadcast operands
                    # measured ~5x slower on HW DVE); tree over u=16 after.
                    pv = Ptv[:, :, 0:nblk * 256].rearrange(
                        "p c (g x) -> p c g x", g=nblk)
                    for g in range(nblk):
                        nc.vector.tensor_tensor(
                            pv[:, :, g, :], win(g), ein(g), MUL)
                    av = Atv[:, :, 0:nblk * 128].rearrange(
                        "p c (g x) -> p c g x", g=nblk)
                    nc.vector.tensor_tensor(
                        av, pv[:, :, :, 0:128], pv[:, :, :, 128:256], ADD)
                    bv = Btv[:, :, 0:nblk * 64].rearrange(
                        "p c (g x) -> p c g x", g=nblk)
                    nc.vector.tensor_tensor(
                        bv, av[:, :, :, 0:64], av[:, :, :, 64:128], ADD)
                    cv = Ctv[:, :, 0:nblk * 32].rearrange(
                        "p c (g x) -> p c g x", g=nblk)
                    nc.vector.tensor_tensor(
                        cv, bv[:, :, :, 0:32], bv[:, :, :, 32:64], ADD)
                    nc.vector.tensor_tensor(
                        tout, cv[:, :, :, 0:16], cv[:, :, :, 16:32], ADD)

                # ---- paths 0,1: contract [sw0|sw1] with s1_0 ----
                T01 = ps.tile([P, C * 32], bf)
                T01v = T01.rearrange("p (c g w) -> p c g w", g=2, w=16)
                e1b = E1.rearrange("p (c x) -> p c x", x=256)
                dve_contract(
                    lambda g: wt[:, :, g * 256:(g + 1) * 256],
                    lambda g: e1b, 2, T01v)

                # ---- path 2: contract sw2 with s1_1[:, :, i] ----
                W2 = wt[:, :, 512:768]
                T2 = ps.tile([P, C * 48], bf)
                T2v = T2.rearrange("p (c i w) -> p c i w", i=3, w=16)
                esb = Es.rearrange("p (c i x) -> p c i x", i=3, x=256)
                dve_contract(
                    lambda g: W2, lambda g: esb[:, :, g, :], 3, T2v)

                # ---- path 4: contract sw4 with s1_1 ----
                W4 = wt[:, :, 1024:1280]
                T4 = ps.tile([P, C * 48], bf)
                T4v = T4.rearrange("p (c i w) -> p c i w", i=3, w=16)
                dve_contract(
                    lambda g: W4, lambda g: esb[:, :, g, :], 3, T4v)

                # ---- path 3: contract sw3 with b. All operands dense
                # (the expansion Eb is already materialized), so this chain
                # runs on DVE at bf16 2x; GPSIMD is the critical path.
                W3 = wt[:, :, 768:1024]
                P3 = pm.tile([P, C * 256], bf)
                P3v = P3.rearrange("p (c x) -> p c x", x=256)
                nc.vector.tensor_tensor(
                    P3v, W3, Eb.rearrange("p (c x) -> p c x", x=256), MUL)
                A3 = pm.tile([P, C * 128], bf)
                A3v = A3.rearrange("p (c x) -> p c x", x=128)
                nc.vector.tensor_tensor(
                    A3v, P3v[:, :, 0:128], P3v[:, :, 128:256], ADD)
                B3 = pm.tile([P, C * 64], bf)
                B3v = B3.rearrange("p (c x) -> p c x", x=64)
                nc.vector.tensor_tensor(
                    B3v, A3v[:, :, 0:64], A3v[:, :, 64:128], ADD)
                C3 = pm.tile([P, C * 32], bf)
                C3v = C3.rearrange("p (c x) -> p c x", x=32)
                nc.vector.tensor_tensor(
                    C3v, B3v[:, :, 0:32], B3v[:, :, 32:64], ADD)
                t3 = ps.tile([P, C * 16], bf)
                t3v = t3.rearrange("p (c w) -> p c w", w=16)
                nc.vector.tensor_tensor(
                    t3v, C3v[:, :, 0:16], C3v[:, :, 16:32], ADD)

                outc = OUTt.rearrange("p (c d) -> p c d", d=64)

                # ---- out0 = t0 * (s2_0*K0) + t3 * K3  (DVE) ----
                o0a = ps.tile([P, C * 16], bf)
                o0av = o0a.rearrange("p (c w) -> p c w", w=16)
                nc.vector.tensor_tensor(
                    o0av, T01v[:, :, 0, :],
                    X2A[:, :].unsqueeze(2).broadcast_to([P, C, 16]), MUL)
                nc.vector.scalar_tensor_tensor(
                    outc[:, :, 0:16], t3v, float(K3), o0av, MUL, ADD)

                # ---- out1 (GPSIMD) ----
                o1a = ps1.tile([P, C * 48], bf)
                o1av = o1a.rearrange("p (c i w) -> p c i w", i=3, w=16)
                nc.gpsimd.tensor_tensor(
                    o1av,
                    T01v[:, :, 1, :].unsqueeze(2).broadcast_to([P, C, 3, 16]),
                    x2bcv[:, :, 1:4].unsqueeze(3).broadcast_to([P, C, 3, 16]),
                    MUL)
                o1b = ps1.tile([P, C * 48], bf)
                o1bv = o1b.rearrange("p (c i w) -> p c i w", i=3, w=16)
                nc.gpsimd.tensor_tensor(
                    o1bv, T2v,
                    x2bcv[:, :, 0].unsqueeze(2).unsqueeze(3)
                        .broadcast_to([P, C, 3, 16]),
                    MUL)
                o1s = ps1.tile([P, C * 48], bf)
                o1sv = o1s.rearrange("p (c i w) -> p c i w", i=3, w=16)
                nc.vector.tensor_tensor(o1sv, o1av, o1bv, ADD)  # dense: DVE

                # cross(T4, s2_1) via duplicated buffers
                T4d = ps1.tile([P, C * 96], bf)
                T4dv = T4d.rearrange("p (c r x) -> p c r x", r=2, x=48)
                nc.gpsimd.tensor_copy(
                    T4dv,
                    T4.rearrange("p (c x) -> p c x", x=48)
                      .unsqueeze(2).broadcast_to([P, C, 2, 48]))
                T4dd = T4d.rearrange("p (c e w) -> p c e w", e=6, w=16)
                x2dd = X2D.rearrange("p (c e) -> p c e", e=6)
                m1 = ps1.tile([P, C * 48], bf)
                m1v = m1.rearrange("p (c i w) -> p c i w", i=3, w=16)
                nc.gpsimd.tensor_tensor(
                    m1v, T4dd[:, :, 1:4, :],
                    x2dd[:, :, 2:5].unsqueeze(3).broadcast_to([P, C, 3, 16]),
                    MUL)
                m2 = ps1.tile([P, C * 48], bf)
                m2v = m2.rearrange("p (c i w) -> p c i w", i=3, w=16)
                nc.gpsimd.tensor_tensor(
                    m2v, T4dd[:, :, 2:5, :],
                    x2dd[:, :, 1:4].unsqueeze(3).broadcast_to([P, C, 3, 16]),
                    MUL)
                crs = ps1.tile([P, C * 48], bf)
                crsv = crs.rearrange("p (c i w) -> p c i w", i=3, w=16)
                nc.vector.tensor_tensor(crsv, m1v, m2v, SUB)  # dense: DVE

                out1ap = outc[:, :, 16:64].rearrange("p c (w i) -> p c i w", i=3)
                nc.gpsimd.tensor_tensor(out1ap, o1sv, crsv, ADD)

                # ---- store ----
                nc.sync.dma_start(outv, OUTt[:])

    nc.compile()
    return nc


_CACHE = {}

# test-harness hooks (ignored by the grading path)
TRACE = False
LAST_RESULTS = None


def _get_nc():
    if "nc" not in _CACHE:
        _CACHE["nc"] = build_bass()
    return _CACHE["nc"]


def kernel(x1, x2, w):
    global LAST_RESULTS
    import ml_dtypes
    from concourse.bass_utils import run_bass_kernel_spmd

    bfnp = ml_dtypes.bfloat16 if USE_BF16 else np.float32
    x1 = np.ascontiguousarray(np.asarray(x1, dtype=np.float32).astype(bfnp))
    x2 = np.ascontiguousarray(np.asarray(x2, dtype=np.float32))
    w = np.ascontiguousarray(np.asarray(w, dtype=np.float32).astype(bfnp))
    z = x1.shape[0]

    pad = Z_PAD - z
    x1p = np.pad(x1, ((0, pad), (0, 0)))
    x2p = np.pad(x2, ((0, pad), (0, 0)))
    wp = np.pad(w, ((0, pad), (0, 0)))

    in_maps = []
    for k in range(N_CORES):
        s = slice(k * ZC, (k + 1) * ZC)
        in_maps.append({
            "x1": np.ascontiguousarray(x1p[s]),
            "x2": np.ascontiguousarray(x2p[s]),
            "w": np.ascontiguousarray(wp[s]),
        })

    nc = _get_nc()
    res = run_bass_kernel_spmd(
        nc, in_maps, core_ids=list(range(N_CORES)), trace=TRACE)
    LAST_RESULTS = res
    out = np.concatenate([r["out"] for r in res.results], axis=0)
    return np.ascontiguousarray(out[:z])



# revision 7
# speedup vs baseline: 1.0839x; 1.0839x over previous
"""Trainium2 Bass kernel for nn_CustomWeightedTensorProduct (e3nn-style weighted
tensor product, 5 paths, per-edge weights).

Strategy (pure data-parallel over the edge/batch dim, 8 cores):
  - Pad Z=100000 -> 100352 = 8 * 12544; each core processes 12544 edges.
  - Per core: 7 tiles of 1792 edges laid out as [128 partitions x 14 edges].
  - Math is factored so each weight element is touched once:
      out0 = (sw0 . s1_0) * s2_0 / sqrt32 + (sw3 . b) / (sqrt3*sqrt32)
               with b[u] = <s1_1[u], s2_1>
      out1[w,i] = ((sw1 . s1_0)[w] * s2_1[i]
                   + (sw2 . s1_1[:,i])[w] * s2_0
                   + cross((sw4 . s1_1), s2_1)[w,i] / sqrt2) / sqrt48
  - Contractions over u run as bf16 tensor_tensor multiplies (DVE 2x mode)
    against Act-engine broadcast-expanded operands, followed by a binary tree
    of bf16 adds (2x mode). Path-3 contraction and the combine stage run on
    GPSIMD; expansions/casts run on the Scalar engine. Final combine in fp32.
"""

import sys

if "/opt/trn_rl_repo" not in sys.path:
    sys.path.insert(0, "/opt/trn_rl_repo")

import numpy as np

Z_FULL = 100000
N_CORES = 8
P = 128
C = 14                      # edges per partition per tile
TILE_E = P * C              # 1792
N_TILES = 7
ZC = TILE_E * N_TILES       # 12544 edges per core
Z_PAD = ZC * N_CORES        # 100352

SQRT2 = 2.0 ** 0.5
SQRT3 = 3.0 ** 0.5
K0 = 1.0 / (32.0 ** 0.5)            # out0 scale
K3 = K0 / SQRT3                     # sw3 path scale
K1 = 1.0 / (48.0 ** 0.5)            # out1 scale
KD = K1 / SQRT2                     # cross path scale

USE_BF16 = True


def build_bass(n_tiles=N_TILES, repeat=1):
    import concourse.bass as bass  # noqa: F401
    import concourse.bacc as bacc
    import concourse.mybir as mybir
    from concourse.tile import TileContext

    zc = TILE_E * n_tiles
    f32 = mybir.dt.float32
    bf = mybir.dt.bfloat16 if USE_BF16 else f32
    ADD = mybir.AluOpType.add
    MUL = mybir.AluOpType.mult
    SUB = mybir.AluOpType.subtract
    AX = mybir.AxisListType.X

    nc = bacc.Bacc(None, target_bir_lowering=False)
    # w/x1 are pre-cast to bf16 on the host: halves HBM traffic and keeps
    # all loads on the fast HWDGE path (no SWDGE cast descriptors).
    x1_d = nc.dram_tensor("x1", [zc, 64], bf, kind="ExternalInput")
    x2_d = nc.dram_tensor("x2", [zc, 4], f32, kind="ExternalInput")
    w_d = nc.dram_tensor("w", [zc, 1280], bf, kind="ExternalInput")
    out_d = nc.dram_tensor("out", [zc, 64], f32, kind="ExternalOutput")

    cast_dma = nc.sync

    with TileContext(nc) as tc:
        with (
            tc.tile_pool(name="io", bufs=2) as pio,
            tc.tile_pool(name="mid", bufs=1) as pm,
            tc.tile_pool(name="small", bufs=2) as ps,
            tc.tile_pool(name="small1", bufs=1) as ps1,
        ):
            for it in range(n_tiles * repeat):
                t = it % n_tiles
                r0 = t * TILE_E
                wv = w_d[r0:r0 + TILE_E, :].rearrange("(p c) d -> p (c d)", p=P)
                x1v = x1_d[r0:r0 + TILE_E, :].rearrange("(p c) d -> p (c d)", p=P)
                x2v = x2_d[r0:r0 + TILE_E, :].rearrange("(p c) d -> p (c d)", p=P)
                outv = out_d[r0:r0 + TILE_E, :].rearrange("(p c) d -> p (c d)", p=P)

                # ---- loads ----
                Wt = pio.tile([P, C * 1280], bf)
                X1t = pio.tile([P, C * 64], bf)
                X2t = pio.tile([P, C * 4], f32)
                OUTt = pio.tile([P, C * 64], f32)
                cast_dma.dma_start(X1t[:], x1v)
                nc.gpsimd.dma_start(X2t[:], x2v)
                cast_dma.dma_start(Wt[:], wv)

                x1b = X1t.rearrange("p (c d) -> p c d", d=64)
                s10 = x1b[:, :, 0:16]                                       # (c,16)
                s11 = x1b[:, :, 16:64].rearrange("p c (u i) -> p c u i", i=3)

                # ---- x2-derived scale tiles. Act handles only contiguous
                # copies (stride-0 reads on Act measured ~13x slow); the
                # broadcast-shaped X2D lives on GPSIMD which handles them fine.
                X2bf = ps.tile([P, C * 4], bf)
                nc.scalar.copy(X2bf[:], X2t[:])
                x2bc = X2bf.rearrange("p (c f) -> p c f", f=4)

                X2A = ps.tile([P, C], bf)       # s2_0 * K0
                nc.scalar.mul(X2A[:], x2bc[:, :, 0], K0)
                X2BC = ps.tile([P, C * 4], bf)  # x2 * K1 (col0=B, cols1:3=C)
                nc.scalar.mul(X2BC[:], X2bf[:], K1)
                x2bcv = X2BC.rearrange("p (c f) -> p c f", f=4)
                X2D = ps.tile([P, C * 6], bf)   # s2_1 * KD, duplicated twice
                nc.gpsimd.tensor_scalar(
                    X2D.rearrange("p (c r k) -> p c r k", r=2, k=3),
                    x2bc[:, :, 1:4].unsqueeze(2).broadcast_to([P, C, 2, 3]),
                    float(KD), None, MUL)

                # ---- b[u] = <s1_1[u,:], s2_1> (DVE mult + reduce) ----
                Bp = pm.tile([P, C * 48], bf)
                Bpv = Bp.rearrange("p (c u i) -> p c u i", u=16, i=3)
                nc.vector.tensor_tensor(
                    Bpv, s11,
                    x2bc[:, :, 1:4].unsqueeze(2).broadcast_to([P, C, 16, 3]), MUL,
                )
                bT = ps.tile([P, C * 16], f32)
                bTv = bT.rearrange("p (c u) -> p c u", u=16)
                nc.vector.tensor_reduce(bTv, Bpv, axis=AX, op=ADD)

                # ---- broadcast expansions on GPSIMD (handles stride-0 reads
                # at ~line rate, unlike Act/DVE) ----
                E1 = pm.tile([P, C * 256], bf)
                E1v = E1.rearrange("p (c u w) -> p c u w", u=16, w=16)
                nc.gpsimd.tensor_copy(
                    E1v, s10.unsqueeze(3).broadcast_to([P, C, 16, 16]))
                Eb = pm.tile([P, C * 256], bf)
                Ebv = Eb.rearrange("p (c u w) -> p c u w", u=16, w=16)
                nc.gpsimd.tensor_copy(
                    Ebv, bTv.unsqueeze(3).broadcast_to([P, C, 16, 16]))
                Es = pm.tile([P, C * 768], bf)
                Esv = Es.rearrange("p (c i u w) -> p c i u w", i=3, u=16, w=16)
                for i in range(3):
                    nc.gpsimd.tensor_copy(
                        Esv[:, :, i],
                        s11[:, :, :, i].unsqueeze(3).broadcast_to([P, C, 16, 16]),
                    )

                wt = Wt.rearrange("p (c q) -> p c q", q=1280)

                # shared DVE scratch
                Pt = pm.tile([P, C * 768], bf)
                Ptv = Pt.rearrange("p (c x) -> p c x", x=768)
                At = pm.tile([P, C * 384], bf)
                Atv = At.rearrange("p (c x) -> p c x", x=384)
                Bt = pm.tile([P, C * 192], bf)
                Btv = Bt.rearrange("p (c x) -> p c x", x=192)
                Ct = pm.tile([P, C * 96], bf)
                Ctv = Ct.rearrange("p (c x) -> p c x", x=96)

                def dve_contract(win, ein, nblk, tout):
                    # Multiply stage split per block so every operand AP is
                    # stride-affine with no zero strides (broadcast operands
                    # measured ~5x slower on HW DVE); tree over u=16 after.
                    pv = Ptv[:, :, 0:nblk * 256].rearrange(
                        "p c (g x) -> p c g x", g=nblk)
                    for g in range(nblk):
                        nc.vector.tensor_tensor(
                            pv[:, :, g, :], win(g), ein(g), MUL)
                    av = Atv[:, :, 0:nblk * 128].rearrange(
                        "p c (g x) -> p c g x", g=nblk)
                    nc.vector.tensor_tensor(
                        av, pv[:, :, :, 0:128], pv[:, :, :, 128:256], ADD)
                    bv = Btv[:, :, 0:nblk * 64].rearrange(
                        "p c (g x) -> p c g x", g=nblk)
                    nc.vector.tensor_tensor(
                        bv, av[:, :, :, 0:64], av[:, :, :, 64:128], ADD)
                    cv = Ctv[:, :, 0:nblk * 32].rearrange(
                        "p c (g x) -> p c g x", g=nblk)
                    nc.vector.tensor_tensor(
                        cv, bv[:, :, :, 0:32], bv[:, :, :, 32:64], ADD)
                    nc.vector.tensor_tensor(
                        tout, cv[:, :, :, 0:16], cv[:, :, :, 16:32], ADD)

                # ---- paths 0,1: contract [sw0|sw1] with s1_0 ----
                T01 = ps.tile([P, C * 32], bf)
                T01v = T01.rearrange("p (c g w) -> p c g w", g=2, w=16)
                e1b = E1.rearrange("p (c x) -> p c x", x=256)
                dve_contract(
                    lambda g: wt[:, :, g * 256:(g + 1) * 256],
                    lambda g: e1b, 2, T01v)

                # ---- path 2: contract sw2 with s1_1[:, :, i] ----
                W2 = wt[:, :, 512:768]
                T2 = ps.tile([P, C * 48], bf)
                T2v = T2.rearrange("p (c i w) -> p c i w", i=3, w=16)
                esb = Es.rearrange("p (c i x) -> p c i x", i=3, x=256)
                dve_contract(
                    lambda g: W2, lambda g: esb[:, :, g, :], 3, T2v)

                # ---- path 4: contract sw4 with s1_1 ----
                W4 = wt[:, :, 1024:1280]
                T4 = ps.tile([P, C * 48], bf)
                T4v = T4.rearrange("p (c i w) -> p c i w", i=3, w=16)
                dve_contract(
                    lambda g: W4, lambda g: esb[:, :, g, :], 3, T4v)

                # ---- path 3 on GPSIMD: contract sw3 with b ----
                W3 = wt[:, :, 768:1024]
                P3 = pm.tile([P, C * 256], bf)
                P3v = P3.rearrange("p (c x) -> p c x", x=256)
                nc.gpsimd.tensor_tensor(
                    P3v, W3, Eb.rearrange("p (c x) -> p c x", x=256), MUL)
                A3 = pm.tile([P, C * 128], bf)
                A3v = A3.rearrange("p (c x) -> p c x", x=128)
                nc.gpsimd.tensor_tensor(
                    A3v, P3v[:, :, 0:128], P3v[:, :, 128:256], ADD)
                B3 = pm.tile([P, C * 64], bf)
                B3v = B3.rearrange("p (c x) -> p c x", x=64)
                nc.gpsimd.tensor_tensor(
                    B3v, A3v[:, :, 0:64], A3v[:, :, 64:128], ADD)
                C3 = pm.tile([P, C * 32], bf)
                C3v = C3.rearrange("p (c x) -> p c x", x=32)
                nc.gpsimd.tensor_tensor(
                    C3v, B3v[:, :, 0:32], B3v[:, :, 32:64], ADD)
                t3 = ps.tile([P, C * 16], bf)
                t3v = t3.rearrange("p (c w) -> p c w", w=16)
                nc.gpsimd.tensor_tensor(
                    t3v, C3v[:, :, 0:16], C3v[:, :, 16:32], ADD)

                outc = OUTt.rearrange("p (c d) -> p c d", d=64)

                # ---- out0 = t0 * (s2_0*K0) + t3 * K3  (DVE) ----
                o0a = ps.tile([P, C * 16], bf)
                o0av = o0a.rearrange("p (c w) -> p c w", w=16)
                nc.vector.tensor_tensor(
                    o0av, T01v[:, :, 0, :],
                    X2A[:, :].unsqueeze(2).broadcast_to([P, C, 16]), MUL)
                nc.vector.scalar_tensor_tensor(
                    outc[:, :, 0:16], t3v, float(K3), o0av, MUL, ADD)

                # ---- out1 (GPSIMD) ----
                o1a = ps1.tile([P, C * 48], bf)
                o1av = o1a.rearrange("p (c i w) -> p c i w", i=3, w=16)
                nc.gpsimd.tensor_tensor(
                    o1av,
                    T01v[:, :, 1, :].unsqueeze(2).broadcast_to([P, C, 3, 16]),
                    x2bcv[:, :, 1:4].unsqueeze(3).broadcast_to([P, C, 3, 16]),
                    MUL)
                o1b = ps1.tile([P, C * 48], bf)
                o1bv = o1b.rearrange("p (c i w) -> p c i w", i=3, w=16)
                nc.gpsimd.tensor_tensor(
                    o1bv, T2v,
                    x2bcv[:, :, 0].unsqueeze(2).unsqueeze(3)
                        .broadcast_to([P, C, 3, 16]),
                    MUL)
                o1s = ps1.tile([P, C * 48], bf)
                o1sv = o1s.rearrange("p (c i w) -> p c i w", i=3, w=16)
                nc.gpsimd.tensor_tensor(o1sv, o1av, o1bv, ADD)

                # cross(T4, s2_1) via duplicated buffers
                T4d = ps1.tile([P, C * 96], bf)
                T4dv = T4d.rearrange("p (c r x) -> p c r x", r=2, x=48)
                nc.gpsimd.tensor_copy(
                    T4dv,
                    T4.rearrange("p (c x) -> p c x", x=48)
                      .unsqueeze(2).broadcast_to([P, C, 2, 48]))
                T4dd = T4d.rearrange("p (c e w) -> p c e w", e=6, w=16)
                x2dd = X2D.rearrange("p (c e) -> p c e", e=6)
                m1 = ps1.tile([P, C * 48], bf)
                m1v = m1.rearrange("p (c i w) -> p c i w", i=3, w=16)
                nc.gpsimd.tensor_tensor(
                    m1v, T4dd[:, :, 1:4, :],
                    x2dd[:, :, 2:5].unsqueeze(3).broadcast_to([P, C, 3, 16]),
                    MUL)
                m2 = ps1.tile([P, C * 48], bf)
                m2v = m2.rearrange("p (c i w) -> p c i w", i=3, w=16)
                nc.gpsimd.tensor_tensor(
                    m2v, T4dd[:, :, 2:5, :],
                    x2dd[:, :, 1:4].unsqueeze(3).broadcast_to([P, C, 3, 16]),
                    MUL)
                crs = ps1.tile([P, C * 48], bf)
                crsv = crs.rearrange("p (c i w) -> p c i w", i=3, w=16)
                nc.gpsimd.tensor_tensor(crsv, m1v, m2v, SUB)

                out1ap = outc[:, :, 16:64].rearrange("p c (w i) -> p c i w", i=3)
                nc.gpsimd.tensor_tensor(out1ap, o1sv, crsv, ADD)

                # ---- store ----
                nc.sync.dma_start(outv, OUTt[:])

    nc.compile()
    return nc


_CACHE = {}

# test-harness hooks (ignored by the grading path)
TRACE = False
LAST_RESULTS = None


def _get_nc():
    if "nc" not in _CACHE:
        _CACHE["nc"] = build_bass()
    return _CACHE["nc"]


def kernel(x1, x2, w):
    global LAST_RESULTS
    import ml_dtypes
    from concourse.bass_utils import run_bass_kernel_spmd

    bfnp = ml_dtypes.bfloat16 if USE_BF16 else np.float32
    x1 = np.ascontiguousarray(np.asarray(x1, dtype=np.float32).astype(bfnp))
    x2 = np.ascontiguousarray(np.asarray(x2, dtype=np.float32))
    w = np.ascontiguousarray(np.asarray(w, dtype=np.float32).astype(bfnp))
    z = x1.shape[0]

    pad = Z_PAD - z
    x1p = np.pad(x1, ((0, pad), (0, 0)))
    x2p = np.pad(x2, ((0, pad), (0, 0)))
    wp = np.pad(w, ((0, pad), (0, 0)))

    in_maps = []
    for k in range(N_CORES):
        s = slice(k * ZC, (k + 1) * ZC)
        in_maps.append({
            "x1": np.ascontiguousarray(x1p[s]),
            "x2": np.ascontiguousarray(x2p[s]),
            "w": np.ascontiguousarray(wp[s]),
        })

    nc = _get_nc()
    res = run_bass_kernel_spmd(
        nc, in_maps, core_ids=list(range(N_CORES)), trace=TRACE)
    LAST_RESULTS = res
    out = np.concatenate([r["out"] for r in res.results], axis=0)
    return np.ascontiguousarray(out[:z])

